# revision 8
# baseline (speedup 1.0000x reference)
"""GraphTransformer (segment-softmax GNN message passing) on 8 TRN2 NeuronCores.

Strategy:
  - Host: sort edges by dst, partition by dst-range (core i owns nodes
    [i*NPC, (i+1)*NPC)) -> complete softmax segments per core, no cross-core
    reduction of messages. Fold weights: W_qk = WQ@WK^T*scale, W_vo = WV@WO.
  - Device per layer: per 128-node dst block, per 128-edge tile:
      gather h[src] rows transposed via dma_gather (fp16, [feat x edge]),
      P2[e,n] = h_srcT^T @ QK_blkT  (logits for all (edge, blk-node) pairs),
      M2 = exp(P2) * onehot(dst_local==n),
      S[n, 0:129] += M2^T @ [h_src | 1]  (weighted scatter + segsum),
    then msg = S[:, :128]/segsum, out = (msg @ W_vo), h = LN(h + out).
  - h exchanged across cores per layer via AllGather (fp16).
"""

import sys
import numpy as np
from contextlib import ExitStack

sys.path.insert(0, "/opt/trn_rl_repo")

import concourse.bacc as bacc
import concourse.bass as bass
import concourse.mybir as mybir
import concourse.tile as tile
from concourse.bass_utils import run_bass_kernel_spmd
from concourse.library_config import mlp

F16 = mybir.dt.float16
F32 = mybir.dt.float32
I16 = mybir.dt.int16
AF = mybir.ActivationFunctionType
ALU = mybir.AluOpType

N_CORES = 8
HID = 128
LN_EPS = 1e-5
PAD_DST = 999.0  # sentinel dst_local for padding edges (no iota match)


def _round_up(x, m):
    return (x + m - 1) // m * m


def build_plan(src, dst, n_nodes):
    """Host preprocessing: per-core edge streams with uniform static shape.

    Returns dict with static plan (shared across cores) + per-core arrays.
    """
    npc = n_nodes // N_CORES
    nblk = (npc + 127) // 128
    order = np.argsort(dst, kind="stable")
    ds = dst[order].astype(np.int64)
    ss = src[order].astype(np.int64)
    cuts = np.searchsorted(ds, np.arange(N_CORES + 1) * npc)

    # per-core edge lists grouped by (block, class); class = src & 3
    per_core = []
    counts = np.zeros((N_CORES, nblk, 4), dtype=np.int64)
    for c in range(N_CORES):
        dc = ds[cuts[c]:cuts[c + 1]] - c * npc
        sc = ss[cuts[c]:cuts[c + 1]]
        blk = dc >> 7
        cls = sc & 3
        key = blk * 4 + cls
        o2 = np.argsort(key, kind="stable")
        dc, sc, key = dc[o2], sc[o2], key[o2]
        counts[c] = np.bincount(key, minlength=nblk * 4).reshape(nblk, 4)
        per_core.append((dc, sc, np.searchsorted(key, np.arange(nblk * 4 + 1))))

    v16 = _round_up(counts.max(axis=0), 16)  # [nblk, 4]
    tot = v16.sum(axis=1)
    k_b = np.maximum(1, (tot + 127) // 128)
    v16[:, 3] += k_b * 128 - tot  # last class absorbs block padding
    offs = np.concatenate([np.zeros((nblk, 1), np.int64), np.cumsum(v16, axis=1)], axis=1)
    L = _round_up(v16, 128)  # declared num_idxs per call (0 if empty)
    W_b = (L.sum(axis=1) // 16).astype(np.int64)   # int16 cols per block
    iw_off = np.concatenate([[0], np.cumsum(W_b)])
    kd_off = np.concatenate([[0], np.cumsum(k_b)])
    IW = int(iw_off[-1])
    KSUM = int(kd_off[-1])
    kmax = int(k_b.max())
    wmax = int(W_b.max())

    idx_all = np.zeros((N_CORES, 128, IW), dtype=np.int16)
    dstl_all = np.full((N_CORES, 128, KSUM), PAD_DST, dtype=np.float32)

    for c in range(N_CORES):
        dc, sc, kcuts = per_core[c]
        for b in range(nblk):
            slot_vals = np.full(int(k_b[b]) * 128, -1, dtype=np.int64)  # src>>2 or pad
            dl_vals = np.full(int(k_b[b]) * 128, PAD_DST, dtype=np.float64)
            colbase = 0
            for j in range(4):
                lo, hi = kcuts[b * 4 + j], kcuts[b * 4 + j + 1]
                n_real = hi - lo
                o = int(offs[b, j])
                slot_vals[o:o + n_real] = sc[lo:hi] >> 2
                slot_vals[o + n_real:o + int(v16[b, j])] = 0  # valid pad idx
                dl_vals[o:o + n_real] = dc[lo:hi] - b * 128
                # pack idx stream for this call
                Lj = int(L[b, j])
                if Lj == 0:
                    continue
                arr = np.full(Lj, -1, dtype=np.int16)
                arr[:int(v16[b, j])] = slot_vals[o:o + int(v16[b, j])]
                packed = arr.reshape(Lj // 16, 16).T  # [16, Lj/16]
                col0 = int(iw_off[b]) + colbase
                idx_all[c, :, col0:col0 + Lj // 16] = np.tile(packed, (8, 1))
                colbase += Lj // 16
            # dst_local layout: [128, k_b], col t = slots t*128..t*128+128
            dstl_all[c, :, int(kd_off[b]):int(kd_off[b]) + int(k_b[b])] = (
                dl_vals.reshape(int(k_b[b]), 128).T.astype(np.float32))

    return dict(
        npc=npc, nblk=nblk, k_b=k_b, v16=v16, L=L, offs=offs,
        iw_off=iw_off, kd_off=kd_off, IW=IW, KSUM=KSUM, kmax=kmax, wmax=wmax,
        idx_all=idx_all, dstl_all=dstl_all,
    )


def build_kernel(plan, n_layers, has_bn, has_bvo, has_lng, has_lnb, has_bout):
    p = plan
    npc, nblk = p["npc"], p["nblk"]
    kmax, wmax = p["kmax"], p["wmax"]
    nsr = npc * N_CORES // 4  # super-rows in gathered h table view [nsr, 512]

    nc = bacc.Bacc("TRN2", target_bir_lowering=False, debug=False,
                   num_devices=N_CORES)

    # ---- I/O ----
    xT = nc.dram_tensor("xT", [128, npc], F16, kind="ExternalInput")
    idx_in = nc.dram_tensor("idx", [128, p["IW"]], I16, kind="ExternalInput")
    dstl_in = nc.dram_tensor("dstl", [128, p["KSUM"]], F32, kind="ExternalInput")
    wqk_in = nc.dram_tensor("wqk", [128, n_layers * 128], F16, kind="ExternalInput")
    wvo_in = nc.dram_tensor("wvo", [128, n_layers * 128], F16, kind="ExternalInput")
    wn_in = nc.dram_tensor("wn", [128, 128], F16, kind="ExternalInput")
    wout_in = nc.dram_tensor("wout", [128, 128], F16, kind="ExternalInput")
    ident_in = nc.dram_tensor("ident", [128, 128], F16, kind="ExternalInput")
    iota_in = nc.dram_tensor("iota", [128, 128], F16, kind="ExternalInput")
    bqk_in = nc.dram_tensor("bqk", [128, n_layers], F32, kind="ExternalInput")
    bn_in = nc.dram_tensor("bn_rep", [128, 128], F32, kind="ExternalInput")
    bvo_in = nc.dram_tensor("bvo_rep", [128, n_layers * 128], F32, kind="ExternalInput")
    lng_in = nc.dram_tensor("lng_rep", [128, n_layers * 128], F32, kind="ExternalInput")
    lnb_in = nc.dram_tensor("lnb_rep", [128, n_layers * 128], F32, kind="ExternalInput")
    bout_in = nc.dram_tensor("bout_rep", [128, 128], F32, kind="ExternalInput")
    out_d = nc.dram_tensor("out", [npc, 128], F32, kind="ExternalOutput")

    ctx = ExitStack()

    with tile.TileContext(nc) as tc:
        nc.gpsimd.load_library(mlp)

        # DRAM intermediates (tile-tracked)
        hsh = [tc.tile([npc, 128], F16, space="DRAM", name=f"hsh{i}")[0]
               for i in range(n_layers + 1)]
        hfl = [tc.tile([nsr, 512], F16, space="DRAM", addr_space="Shared",
                       name=f"hfl{i}")[0] for i in range(2)]

        # ---- constants to SBUF ----
        with tc.tile_pool(name="const", bufs=1) as cpool:
            def cload(dram, shape, dt, nm):
                t = cpool.tile(shape, dt, name=nm)
                nc.sync.dma_start(t[:, :], dram[:, :])
                return t
            wqk_sb = cload(wqk_in, [128, n_layers * 128], F16, "wqk_sb")
            wvo_sb = cload(wvo_in, [128, n_layers * 128], F16, "wvo_sb")
            wn_sb = cload(wn_in, [128, 128], F16, "wn_sb")
            wout_sb = cload(wout_in, [128, 128], F16, "wout_sb")
            ident_sb = cload(ident_in, [128, 128], F16, "ident_sb")
            iota_sb = cload(iota_in, [128, 128], F16, "iota_sb")
            bqk_sb = cload(bqk_in, [128, n_layers], F32, "bqk_sb")
            eps_sb = cpool.tile([128, 1], F32, name="eps_sb")
            nc.vector.memset(eps_sb[:, :], LN_EPS)
            bn_sb = cload(bn_in, [128, 128], F32, "bn_sb") if has_bn else None
            bvo_sb = cload(bvo_in, [128, n_layers * 128], F32, "bvo_sb") if has_bvo else None
            lng_sb = cload(lng_in, [128, n_layers * 128], F32, "lng_sb") if has_lng else None
            lnb_sb = cload(lnb_in, [128, n_layers * 128], F32, "lnb_sb") if has_lnb else None
            bout_sb = cload(bout_in, [128, 128], F32, "bout_sb") if has_bout else None

            with (
                tc.tile_pool(name="hblk", bufs=3) as hblk_pool,
                tc.tile_pool(name="gath", bufs=3) as gath_pool,
                tc.tile_pool(name="meta", bufs=3) as meta_pool,
                tc.tile_pool(name="work", bufs=4) as work,
                tc.tile_pool(name="blk", bufs=3) as blk_sb,
                tc.tile_pool(name="psA", bufs=2, space="PSUM") as psA,
                tc.tile_pool(name="psB", bufs=2, space="PSUM") as psB,
                tc.tile_pool(name="psS", bufs=2, space="PSUM") as psS,
                tc.tile_pool(name="psC", bufs=2, space="PSUM") as psC,
            ):
                def MM(out, lhsT, rhs, **kw):
                    nc.tensor.matmul(out, lhsT, rhs, **kw)

                def TR(out, in_, nn):
                    nc.tensor.transpose(out, in_, ident_sb[:nn, :nn])

                def blocks():
                    for b in range(nblk):
                        n = min(128, npc - b * 128)
                        yield b, n, b * 128

                # ======== embed: h0 = x @ Wn (+ bn) ========
                for b, n, r0 in blocks():
                    xt = hblk_pool.tile([128, 128], F16, tag="xt")
                    nc.sync.dma_start(xt[:, :n], xT[:, r0:r0 + n])
                    h0p = psC.tile([128, 128], F32, tag="pc")
                    MM(h0p[:n, :], xt[:, :n], wn_sb[:, :])
                    h016 = blk_sb.tile([128, 128], F16, tag="hn16")
                    if has_bn:
                        r32 = blk_sb.tile([128, 128], F32, tag="res")
                        nc.vector.tensor_tensor(r32[:n, :], h0p[:n, :], bn_sb[:n, :], ALU.add)
                        nc.scalar.activation(h016[:n, :], r32[:n, :], AF.Copy)
                    else:
                        nc.scalar.activation(h016[:n, :], h0p[:n, :], AF.Copy)
                    nc.sync.dma_start(hsh[0][r0:r0 + n, :], h016[:n, :])

                ag_inst = nc.gpsimd.collective_compute(
                    "AllGather", ALU.bypass,
                    replica_groups=[list(range(N_CORES))],
                    ins=[hsh[0].opt()], outs=[hfl[0].opt()])

                # ======== layers ========
                for l in range(n_layers):
                    hful = hfl[l % 2]
                    ls = slice(l * 128, (l + 1) * 128)
                    for b, n, r0 in blocks():
                        kb = int(p["k_b"][b])
                        # metadata
                        wb = int(p["iw_off"][b + 1] - p["iw_off"][b])
                        idxt = meta_pool.tile([128, wmax], I16, tag="idx")
                        nc.sync.dma_start(
                            idxt[:, :wb],
                            idx_in[:, int(p["iw_off"][b]):int(p["iw_off"][b]) + wb])
                        dstt = meta_pool.tile([128, kmax], F32, tag="dst")
                        nc.sync.dma_start(
                            dstt[:, :kb],
                            dstl_in[:, int(p["kd_off"][b]):int(p["kd_off"][b]) + kb])
                        # h_blk -> QK_blkT
                        hb16 = hblk_pool.tile([128, 128], F16, tag="hb16")
                        nc.sync.dma_start(hb16[:n, :], hsh[l][r0:r0 + n, :])
                        trh = psB.tile([128, 128], F16, tag="tp")
                        TR(trh[:, :n], hb16[:n, :], n)
                        hbT = blk_sb.tile([128, 128], F16, tag="hbT")
                        nc.vector.tensor_copy(hbT[:, :n], trh[:, :n])
                        qkp = psC.tile([128, 128], F32, tag="pc")
                        MM(qkp[:, :n], wqk_sb[:, ls], hbT[:, :n])
                        qkT = blk_sb.tile([128, 128], F16, tag="qkT")
                        nc.vector.tensor_scalar(
                            qkT[:, :n], qkp[:, :n], bqk_sb[:, l:l + 1], None, ALU.add)
                        # gathers (4 classes)
                        gb = gath_pool.tile([128, 1, kmax * 128 + 128], F16, tag="gb")
                        colbase = 0
                        off = 0
                        for j in range(4):
                            Lj = int(p["L"][b, j])
                            vj = int(p["v16"][b, j])
                            if Lj == 0:
                                continue
                            gi = nc.gpsimd.dma_gather(
                                gb[:, :, off:off + Lj],
                                hful[:, j * 128:(j + 1) * 128],
                                idxt[:, colbase:colbase + Lj // 16],
                                Lj, vj, 128, elem_step=512, transpose=True)
                            tile.add_dep_helper(
                                gi.ins if hasattr(gi, "ins") else gi,
                                ag_inst.ins if hasattr(ag_inst, "ins") else ag_inst,
                                reason="gather after AG")
                            off += vj
                            colbase += Lj // 16
                        # edge tiles
                        Sp = psS.tile([128, 129], F32, tag="S")
                        for t in range(kb):
                            cs = slice(t * 128, (t + 1) * 128)
                            tp = psB.tile([128, 128], F16, tag="tp")
                            TR(tp[:, :], gb[:, 0, cs], 128)
                            he = work.tile([128, 129], F16, tag="he")
                            nc.vector.tensor_copy(he[:, :128], tp[:, :])
                            nc.vector.memset(he[:, 128:129], 1.0)
                            p2 = psA.tile([128, 128], F32, tag="p2")
                            MM(p2[:, :n], gb[:, 0, cs], qkT[:, :n])
                            m2r = work.tile([128, 128], F16, tag="m2r")
                            nc.scalar.activation(m2r[:, :n], p2[:, :n], AF.Exp)
                            h2 = work.tile([128, 128], F16, tag="h2")
                            nc.vector.tensor_scalar(
                                h2[:, :n], iota_sb[:, :n], dstt[:, t:t + 1], None,
                                ALU.is_equal)
                            m2 = work.tile([128, 128], F16, tag="m2")
                            nc.vector.tensor_tensor(
                                m2[:, :n], m2r[:, :n], h2[:, :n], ALU.mult)
                            MM(Sp[:n, :], m2[:, :n], he[:, :],
                               start=(t == 0), stop=(t == kb - 1))
                        # block epilogue
                        seg = blk_sb.tile([128, 1], F32, tag="seg")
                        nc.vector.tensor_scalar(
                            seg[:n, :], Sp[:n, 128:129], 1e-30, None, ALU.max)
                        rec = blk_sb.tile([128, 1], F32, tag="rec")
                        nc.vector.reciprocal(rec[:n, :], seg[:n, :])
                        sn = blk_sb.tile([128, 128], F16, tag="sn")
                        nc.vector.tensor_scalar(
                            sn[:n, :], Sp[:n, :128], rec[:n, :], None, ALU.mult)
                        trs = psB.tile([128, 128], F16, tag="tp")
                        TR(trs[:, :n], sn[:n, :], n)
                        snT = blk_sb.tile([128, 128], F16, tag="snT")
                        nc.vector.tensor_copy(snT[:, :n], trs[:, :n])
                        op = psC.tile([128, 128], F32, tag="pc")
                        MM(op[:n, :], snT[:, :n], wvo_sb[:, ls])
                        res = blk_sb.tile([128, 128], F32, tag="res")
                        nc.vector.tensor_tensor(
                            res[:n, :], op[:n, :], hb16[:n, :], ALU.add)
                        if has_bvo:
                            nc.vector.tensor_tensor(
                                res[:n, :], res[:n, :], bvo_sb[:n, ls], ALU.add)
                        # layernorm along free axis
                        sm = blk_sb.tile([128, 1], F32, tag="sm")
                        nc.vector.tensor_reduce(sm[:n, :], res[:n, :], mybir.AxisListType.X, ALU.add)
                        mu = blk_sb.tile([128, 1], F32, tag="mu")
                        nc.vector.tensor_scalar(
                            mu[:n, :], sm[:n, :], 1.0 / 128, None, ALU.mult)
                        xm = blk_sb.tile([128, 128], F32, tag="xm")
                        nc.vector.tensor_scalar(
                            xm[:n, :], res[:n, :], mu[:n, :], None, ALU.subtract)
                        sq = blk_sb.tile([128, 128], F32, tag="sq")
                        ssq = blk_sb.tile([128, 1], F32, tag="ssq")
                        nc.scalar.activation(sq[:n, :], xm[:n, :], AF.Square,
                                             accum_out=ssq[:n, :])
                        std = blk_sb.tile([128, 1], F32, tag="std")
                        nc.scalar.activation(std[:n, :], ssq[:n, :], AF.Sqrt,
                                             bias=eps_sb[:n, :], scale=1.0 / 128)
                        rst = blk_sb.tile([128, 1], F32, tag="rst")
                        nc.vector.reciprocal(rst[:n, :], std[:n, :])
                        if has_lng or has_lnb:
                            xn = blk_sb.tile([128, 128], F32, tag="xn")
                            nc.vector.tensor_scalar(
                                xn[:n, :], xm[:n, :], rst[:n, :], None, ALU.mult)
                            if has_lng:
                                nc.vector.tensor_tensor(
                                    xn[:n, :], xn[:n, :], lng_sb[:n, ls], ALU.mult)
                            if has_lnb:
                                nc.vector.tensor_tensor(
                                    xn[:n, :], xn[:n, :], lnb_sb[:n, ls], ALU.add)
                            hn = blk_sb.tile([128, 128], F16, tag="hn16")
                            nc.scalar.activation(hn[:n, :], xn[:n, :], AF.Copy)
                        else:
                            hn = blk_sb.tile([128, 128], F16, tag="hn16")
                            nc.vector.tensor_scalar(
                                hn[:n, :], xm[:n, :], rst[:n, :], None, ALU.mult)
                        if l < n_layers - 1:
                            nc.sync.dma_start(hsh[l + 1][r0:r0 + n, :], hn[:n, :])
                        else:
                            trf = psB.tile([128, 128], F16, tag="tp")
                            TR(trf[:, :n], hn[:n, :], n)
                            hfT = blk_sb.tile([128, 128], F16, tag="hfT")
                            nc.vector.tensor_copy(hfT[:, :n], trf[:, :n])
                            fo = psC.tile([128, 128], F32, tag="pc")
                            MM(fo[:n, :], hfT[:, :n], wout_sb[:, :])
                            o32 = blk_sb.tile([128, 128], F32, tag="o32")
                            if has_bout:
                                nc.vector.tensor_tensor(
                                    o32[:n, :], fo[:n, :], bout_sb[:n, :], ALU.add)
                            else:
                                nc.vector.tensor_copy(o32[:n, :], fo[:n, :])
                            nc.sync.dma_start(out_d[r0:r0 + n, :], o32[:n, :])
                    if l < n_layers - 1:
                        ag_inst = nc.gpsimd.collective_compute(
                            "AllGather", ALU.bypass,
                            replica_groups=[list(range(N_CORES))],
                            ins=[hsh[l + 1].opt()], outs=[hfl[(l + 1) % 2].opt()])

    nc.compile()
    return nc


_CACHE = {}


def kernel(x, edge_index, Wn, bn, WQ, bQ, WK, bK, WV, bV, WO, bO,
           ln_g, ln_b, Wout, bout):
    x = np.asarray(x)
    edge_index = np.asarray(edge_index)
    n_nodes, node_dim = x.shape
    n_layers = np.asarray(WQ).shape[0]
    assert node_dim == HID
    scale = 1.0 / np.sqrt(HID)

    src = np.asarray(edge_index[0]).astype(np.int64)
    dst = np.asarray(edge_index[1]).astype(np.int64)

    plan = build_plan(src, dst, n_nodes)
    npc = plan["npc"]

    # folded weights (fp32 math, fp16 storage)
    WQ, bQ = np.asarray(WQ, np.float32), np.asarray(bQ, np.float32)
    WK, WV = np.asarray(WK, np.float32), np.asarray(WV, np.float32)
    WO, bO = np.asarray(WO, np.float32), np.asarray(bO, np.float32)
    bV = np.asarray(bV, np.float32)
    w_qk = np.stack([WQ[l] @ WK[l].T * scale for l in range(n_layers)])  # [L,fj,fi]
    b_qk = np.stack([bQ[l] @ WK[l].T * scale for l in range(n_layers)])  # [L,fi]
    w_vo = np.stack([WV[l] @ WO[l] for l in range(n_layers)])            # [L,fi,fo]
    b_vo = np.stack([bV[l] @ WO[l] + bO[l] for l in range(n_layers)])    # [L,fo]
    ln_g = np.asarray(ln_g, np.float32)
    ln_b = np.asarray(ln_b, np.float32)
    bn = np.asarray(bn, np.float32)
    bout = np.asarray(bout, np.float32)

    has_bn = bool(np.any(bn))
    has_bvo = bool(np.any(b_vo))
    has_lng = bool(np.any(ln_g != 1.0))
    has_lnb = bool(np.any(ln_b))
    has_bout = bool(np.any(bout))

    key = (n_nodes, src.shape[0], n_layers)
    if key not in _CACHE:
        _CACHE[key] = build_kernel(plan, n_layers, has_bn, has_bvo,
                                   has_lng, has_lnb, has_bout)
    nc = _CACHE[key]

    # shared (per-core identical) input tensors
    wqk_h = np.concatenate([w_qk[l] for l in range(n_layers)], axis=1).astype(np.float16)
    wvo_h = np.concatenate([w_vo[l] for l in range(n_layers)], axis=1).astype(np.float16)
    bqk_h = np.stack([b_qk[l] for l in range(n_layers)], axis=1).astype(np.float32)
    shared = {
        "wqk": wqk_h, "wvo": wvo_h,
        "wn": np.asarray(Wn, np.float32).astype(np.float16),
        "wout": np.asarray(Wout, np.float32).astype(np.float16),
        "ident": np.eye(128, dtype=np.float16),
        "iota": np.tile(np.arange(128, dtype=np.float16)[None, :], (128, 1)),
        "bqk": bqk_h,
        "bn_rep": np.tile(bn[None, :], (128, 1)).astype(np.float32),
        "bvo_rep": np.concatenate(
            [np.tile(b_vo[l][None, :], (128, 1)) for l in range(n_layers)],
            axis=1).astype(np.float32),
        "lng_rep": np.concatenate(
            [np.tile(ln_g[l][None, :], (128, 1)) for l in range(n_layers)],
            axis=1).astype(np.float32),
        "lnb_rep": np.concatenate(
            [np.tile(ln_b[l][None, :], (128, 1)) for l in range(n_layers)],
            axis=1).astype(np.float32),
        "bout_rep": np.tile(bout[None, :], (128, 1)).astype(np.float32),
    }
    x32 = np.asarray(x, np.float32)
    in_maps = []
    for c in range(N_CORES):
        m = dict(shared)
        m["xT"] = np.ascontiguousarray(x32[c * npc:(c + 1) * npc].T).astype(np.float16)
        m["idx"] = np.ascontiguousarray(plan["idx_all"][c])
        m["dstl"] = np.ascontiguousarray(plan["dstl_all"][c])
        in_maps.append(m)

    global _LAST_RES
    want_trace = globals().get("_WANT_TRACE", False)
    res = run_bass_kernel_spmd(nc, in_maps, core_ids=list(range(N_CORES)),
                               trace=want_trace)
    _LAST_RES = res
    outs = [r["out"] for r in res.results]
    return np.concatenate(outs, axis=0).astype(np.float32)


if __name__ == "__main__":
    # mini smoke test vs numpy reference
    rng = np.random.default_rng(0)
    N, E, L = 2048, 8192, 4
    x = rng.normal(size=(N, HID)).astype(np.float32)
    ei = rng.integers(0, N, size=(2, E)).astype(np.int64)
    s = 1.0 / np.sqrt(HID)
    inp = dict(
        x=x, edge_index=ei,
        Wn=(rng.normal(size=(HID, HID)) / np.sqrt(HID)).astype(np.float32),
        bn=np.zeros(HID, np.float32),
        WQ=(rng.normal(size=(L, HID, HID)) * s).astype(np.float32),
        bQ=np.zeros((L, HID), np.float32),
        WK=(rng.normal(size=(L, HID, HID)) * s).astype(np.float32),
        bK=np.zeros((L, HID), np.float32),
        WV=(rng.normal(size=(L, HID, HID)) * s).astype(np.float32),
        bV=np.zeros((L, HID), np.float32),
        WO=(rng.normal(size=(L, HID, HID)) * s).astype(np.float32),
        bO=np.zeros((L, HID), np.float32),
        ln_g=np.ones((L, HID), np.float32),
        ln_b=np.zeros((L, HID), np.float32),
        Wout=(rng.normal(size=(HID, HID)) * s).astype(np.float32),
        bout=np.zeros(HID, np.float32),
    )

    def ref(x, edge_index, Wn, bn, WQ, bQ, WK, bK, WV, bV, WO, bO, ln_g, ln_b,
            Wout, bout):
        src, dstv = edge_index[0], edge_index[1]
        h = x @ Wn + bn
        for l in range(L):
            Q = h @ WQ[l] + bQ[l]
            K = h @ WK[l] + bK[l]
            V = h @ WV[l] + bV[l]
            sc = (Q[dstv] * K[src]).sum(-1) * s
            e = np.exp(sc)
            segs = np.zeros(N); np.add.at(segs, dstv, e)
            msg = np.zeros((N, HID))
            np.add.at(msg, dstv, e[:, None] * V[src])
            msg = msg / np.maximum(segs, 1e-30)[:, None]
            o = msg @ WO[l] + bO[l]
            r = h + o
            muv = r.mean(-1, keepdims=True)
            va = ((r - muv) ** 2).mean(-1, keepdims=True)
            h = (r - muv) / np.sqrt(va + LN_EPS) * ln_g[l] + ln_b[l]
        return h @ Wout + bout

    expected = ref(**inp)
    actual = kernel(**inp)
    err = np.abs(actual - expected) / (np.abs(expected).mean() + 1e-9)
    print("mini rel err (mean abs / mean |ref|):", err.mean(), "max:", err.max())
